# revision 24
# baseline (speedup 1.0000x reference)
"""Trainium2 Bass kernel for BaseGCN graph Laplacian (B=4, N=4096, C=3, k=20).

Math: reference computes L = I - D^{-1/2} A D^{-1/2} with A the one-hot
scatter of the k=20 nearest neighbours (euclidean, self included) per row.
top_k always returns exactly k distinct indices, so deg == k for every row
and L = I - A/k exactly: 0.95 on the diagonal, -0.05 at the 19 non-self
neighbour columns, 0 elsewhere.

Sharding: 8 cores; core = 2*b + half owns rows [half*2048, half*2048+2048)
of batch b and emits a (2048, 4096) f32 output slice.

Device algorithm per 128-row chunk (target: the 2 MiB/chunk output DMA at
the ~358 GB/s HBM-per-core limit, ~5.9us, sets the pace; DVE is trimmed to
~6.5us by doing the row scan in fp16 and the compare in a 4x DVE mode):
  PE    s[i,j] = 2<x_i,x_j> - sq_j - (sq_i + 1e-3) via a K=24 bf16-limb
        matmul (limb products exact in fp32, error ~2^-26 x^2) -> PSUM f32.
        The 1e-3 row shift makes every s strictly negative (s_ii = -1e-3
        >> the ~1e-5 limb rounding), which makes the fp16 bit pattern
        order-monotone: for same-sign IEEE values, value order == bit
        order for ANY exponent/mantissa split, so fp16 bytes can be
        compared by bf16-typed ops.
  ACT   copies PSUM -> SBUF rounding to fp16 *bytes* (the SBUF tile is
        declared bf16; the activation out AP is bitcast to f16 - the ISA
        encoding is identical, only fp16 resolution is what matters:
        fp16 ulp near the threshold is ~2% of the neighbour gap).
  DVE   scan: 8 x max8 over 512-col segments (fp16-typed APs) -> 64
        candidates, 3 rounds max8+match_replace -> T = 20th largest
        (exact unless >=9 of a row's top-20 land in one segment), one
        [128,1] copy of T's bytes bf16->f32, then ONE all-bf16-typed
        tensor_scalar (s_bits >= T_bits) * (-1/k) over the full 4096
        columns in 4x DVE perf mode (~1.2us; measured 629ns/2048 cols).
  GPSIMD only tiny work - identity add at the diagonal block (identc
        input carries the per-core eye position so one NEFF serves all
        cores) and the output DMA emission: SWDGE dma_start casts the
        bf16-valued {0, -0.05, 0.95} tile to f32 DRAM, halving SBUF-side
        traffic while HBM receives the full f32 output. Big GpSimd tensor
        ops are deliberately avoided: concurrent GpSimd/DVE SBUF access
        degrades both ~25x (shared SBUF ports).
Accuracy (measured offline on the real inputs): ~245 flipped +-0.05
entries -> rel err ~6.3e-3 vs the 2e-2 gate.
"""

import numpy as np

B, N, C = 4, 4096, 3
K = 20
P = 128                     # partition rows per chunk
ROWS = N // 2               # rows per core
NCHUNK = ROWS // P          # 16
NSEG = 8                    # 512-col max8 segments per row
SEGW = N // NSEG
NEG16 = -60000.0            # candidate kill value (fp16-representable)
SHIFT = 1e-3                # strict-negativity shift on s (see docstring)
# Match the reference's fl(dinv*dinv) rounding: dinv = fl(1/sqrt(20)) in f32.
_DINV = np.float32(1.0) / np.sqrt(np.float32(K))
VNEIGH = -float(np.float32(_DINV * _DINV))

_NC_CACHE = []


KMM = 24  # bf16-limb contraction depth


def _build_bass():
    import concourse.mybir as mybir
    import concourse.tile as tile
    from concourse import bacc

    f32 = mybir.dt.float32
    f16 = mybir.dt.float16
    bf16 = mybir.dt.bfloat16
    ge_mult = dict(op0=mybir.AluOpType.is_ge, op1=mybir.AluOpType.mult)
    nc = bacc.Bacc("TRN2", debug=False, num_devices=8)
    rh = nc.dram_tensor("rh", (KMM, N), bf16, kind="ExternalInput").ap()
    lh = nc.dram_tensor("lh", (KMM, ROWS), bf16, kind="ExternalInput").ap()
    identc = nc.dram_tensor("identc", (P, 2 * P), bf16, kind="ExternalInput").ap()
    outp = nc.dram_tensor("outp", (ROWS, N), f32, kind="ExternalOutput").ap()

    with tile.TileContext(nc) as tc:
        with (
            tc.tile_pool(name="const", bufs=1) as const_pool,
            tc.tile_pool(name="psum", bufs=2, space="PSUM") as psum_pool,
            tc.tile_pool(name="sbig", bufs=6) as s_pool,
            # Deep small-tile buffering: the per-chunk cand/m/t32 recycle
            # barrier otherwise dwells ~640ns/chunk in the DVE stream.
            tc.tile_pool(name="small", bufs=8) as small_pool,
            # 8 output buffers: the ~5us/chunk output DMA (+~0.9us sem
            # propagation) must never make the DVE wait to reuse a tile.
            tc.tile_pool(name="outt", bufs=8) as out_pool,
        ):
            # Stage the input DMAs so chunk 0's first matmul (which reads
            # lh[:, :128] and rh[:, :512]) can start as soon as those small
            # pieces land, ahead of the bulk (Tile tracks sub-tile ranges).
            rh_sb = const_pool.tile([KMM, N], bf16)
            lh_sb = const_pool.tile([KMM, ROWS], bf16)
            id_sb = const_pool.tile([P, 2 * P], bf16)
            nc.sync.dma_start(lh_sb[:, 0:P], lh[:, 0:P])
            nc.sync.dma_start(rh_sb[:, 0:512], rh[:, 0:512])
            nc.sync.dma_start(rh_sb[:, 512:1024], rh[:, 512:1024])
            nc.sync.dma_start(rh_sb[:, 1024:2048], rh[:, 1024:2048])
            nc.sync.dma_start(rh_sb[:, 2048:N], rh[:, 2048:N])
            nc.scalar.dma_start(lh_sb[:, P:ROWS], lh[:, P:ROWS])
            nc.scalar.dma_start(id_sb[:], identc)

            for c in range(NCHUNK):
                # s16 holds fp16 BYTES; declared bf16 so the compare can
                # read it with plain (fast-mode) bf16 APs.
                s16 = s_pool.tile([P, N], bf16, tag="s16")
                for h in range(2):
                    for t in range(4):
                        # One [128,512] PSUM tile per matmul (4 tags x
                        # bufs=2 x 2KB = the full 16KB PSUM): each ACT
                        # piece depends on a single matmul, so the copy
                        # (and chunk 0's scan) trails the PE closely.
                        ps = psum_pool.tile([P, 512], f32, tag=f"ps{t}")
                        col = h * (N // 2) + t * 512
                        nc.tensor.matmul(
                            ps[:],
                            lh_sb[:, c * P:(c + 1) * P],
                            rh_sb[:, col:col + 512],
                            start=True,
                            stop=True,
                        )
                        nc.scalar.activation(
                            s16[:, col:col + 512].bitcast(f16),
                            ps[:],
                            mybir.ActivationFunctionType.Copy,
                        )

                cand = small_pool.tile([P, NSEG * 8], bf16, tag="cand")
                for g in range(NSEG):
                    nc.vector.max(cand[:, g * 8:(g + 1) * 8].bitcast(f16),
                                  s16[:, g * SEGW:(g + 1) * SEGW].bitcast(f16))
                m = small_pool.tile([P, 24], bf16, tag="m")
                cd, mm = cand[:].bitcast(f16), m[:].bitcast(f16)
                nc.vector.max(mm[:, 0:8], cd)
                nc.vector.match_replace(cd, mm[:, 0:8], cd, NEG16)
                nc.vector.max(mm[:, 8:16], cd)
                nc.vector.match_replace(cd, mm[:, 8:16], cd, NEG16)
                nc.vector.max(mm[:, 16:24], cd)
                # T = 20th largest = index 19. Read its BYTES as bf16 and
                # widen to f32: the compare runs entirely in bf16-bit space
                # (order-equivalent to fp16 value space; all s < 0).
                t32 = small_pool.tile([P, 1], f32, tag="t32")
                nc.vector.tensor_copy(t32[:], m[:, 19:20])

                ot = out_pool.tile([P, N], bf16, tag="ot")
                dcols = [(c * P, id_sb[:, 0:P]), (ROWS + c * P, id_sb[:, P:2 * P])]
                if c < 2:
                    # Head chunks: 2 pieces so the first output bytes hit
                    # the (otherwise idle) DMA queue ~5us earlier.
                    for p0 in (0, N // 2):
                        qs = slice(p0, p0 + N // 2)
                        nc.vector.tensor_scalar(
                            ot[:, qs], s16[:, qs], t32[:], VNEIGH, **ge_mult)
                        for dcol, idslice in dcols:
                            if p0 <= dcol < p0 + N // 2:
                                nc.gpsimd.tensor_add(
                                    ot[:, dcol:dcol + P], ot[:, dcol:dcol + P],
                                    idslice)
                        nc.gpsimd.dma_start(outp[c * P:(c + 1) * P, qs], ot[:, qs])
                elif c < NCHUNK - 1:
                    nc.vector.tensor_scalar(
                        ot[:], s16[:], t32[:], VNEIGH, **ge_mult)
                    for dcol, idslice in dcols:
                        nc.gpsimd.tensor_add(
                            ot[:, dcol:dcol + P], ot[:, dcol:dcol + P], idslice)
                    nc.gpsimd.dma_start(outp[c * P:(c + 1) * P, :], ot[:])
                else:
                    # Tail: piecewise compare+DMA so the final transfer is
                    # only 256 KB; diag-bearing pieces first; eye adds on
                    # DVE (tiny) so GpSimd only emits DMA pieces.
                    pieces = [(3072, 1024), (1024, 1024), (0, 1024),
                              (2048, 512), (2560, 512)]
                    for p0, pw in pieces:
                        qs = slice(p0, p0 + pw)
                        nc.vector.tensor_scalar(
                            ot[:, qs], s16[:, qs], t32[:], VNEIGH, **ge_mult)
                        for dcol, idslice in dcols:
                            if p0 <= dcol < p0 + pw:
                                nc.vector.tensor_add(
                                    ot[:, dcol:dcol + P], ot[:, dcol:dcol + P],
                                    idslice)
                        nc.gpsimd.dma_start(outp[c * P:(c + 1) * P, qs], ot[:, qs])
    nc.compile()
    return nc


def _split3(v):
    """Split fp32 array into three bf16 limbs: v ~= h + m + l (24 bits)."""
    import ml_dtypes

    bf = ml_dtypes.bfloat16
    h = v.astype(bf)
    r = (v - h.astype(np.float32)).astype(np.float32)
    m = r.astype(bf)
    l = (r - m.astype(np.float32)).astype(bf)
    return h, m, l


def _make_in_maps(x):
    import ml_dtypes

    bf = ml_dtypes.bfloat16
    eye = np.eye(P, dtype=np.float32).astype(bf)
    zero = np.zeros((P, P), dtype=np.float32).astype(bf)
    in_maps = []
    for core in range(8):
        b, half = divmod(core, 2)
        xb = x[b]                                            # (N, C)
        sq = (xb * xb).sum(axis=1, dtype=np.float32)
        rows = slice(half * ROWS, (half + 1) * ROWS)
        rh = np.empty((KMM, N), bf)
        lhs = np.empty((KMM, ROWS), bf)
        for c in range(3):
            h, m, l = _split3(xb[:, c])
            h2 = (2.0 * h.astype(np.float32)).astype(bf)
            m2 = (2.0 * m.astype(np.float32)).astype(bf)
            l2 = (2.0 * l.astype(np.float32)).astype(bf)
            # product pairs (lhs, rhs): (2h,h) (2h,m) (2m,h) (2m,m) (2h,l) (2l,h)
            rh[6 * c + 0] = h
            rh[6 * c + 1] = m
            rh[6 * c + 2] = h
            rh[6 * c + 3] = m
            rh[6 * c + 4] = l
            rh[6 * c + 5] = h
            lhs[6 * c + 0] = h2[rows]
            lhs[6 * c + 1] = h2[rows]
            lhs[6 * c + 2] = m2[rows]
            lhs[6 * c + 3] = m2[rows]
            lhs[6 * c + 4] = h2[rows]
            lhs[6 * c + 5] = l2[rows]
        sh, sm, sl = _split3(sq)
        # -sq_j rows: lhs = -1, rhs = sq limbs
        rh[18], rh[19], rh[20] = sh, sm, sl
        lhs[18] = lhs[19] = lhs[20] = np.array(-1.0, bf)
        # -(sq_i + SHIFT) rows: lhs = -limbs(sq + SHIFT), rhs = 1. The
        # shift keeps every s strictly negative (bit-order trick).
        ssh, ssm, ssl = _split3(sq + np.float32(SHIFT))
        rh[21] = rh[22] = rh[23] = np.array(1.0, bf)
        lhs[21] = (-ssh.astype(np.float32)).astype(bf)[rows]
        lhs[22] = (-ssm.astype(np.float32)).astype(bf)[rows]
        lhs[23] = (-ssl.astype(np.float32)).astype(bf)[rows]
        identc = np.ascontiguousarray(
            np.concatenate([eye, zero] if half == 0 else [zero, eye], axis=1)
        )
        in_maps.append({"rh": rh, "lh": lhs, "identc": identc})
    return in_maps


def _ensure_trace_safe():
    """run_bass_kernel_spmd(trace=True) (e.g. env BASS_TRACE=1) needs
    antenv.axon_hooks, which some images lack, and an artifact upload that
    needs bucket access. Stub both so a traced run degrades instead of
    crashing; with tracing off these are unused. When the image lacks
    antenv.axon_hooks, the sitecustomize boot skipped NTFF-hook
    registration (ImportError -> silent pass), so after stubbing we
    re-run that registration step ourselves via trn_agent_boot."""
    import sys
    import types

    try:
        import antenv.axon_hooks  # noqa: F401
    except Exception:
        m = types.ModuleType("antenv.axon_hooks")
        m._H = None
        m.set_axon_ntff_profile_hook = lambda h: setattr(m, "_H", h)
        m.get_axon_ntff_profile_hook = lambda: m._H
        sys.modules["antenv.axon_hooks"] = m
        try:
            import antenv

            antenv.axon_hooks = m
        except Exception:
            pass
        try:
            from trn_agent_boot import trn_boot

            hook = trn_boot._ntff_profile_via_ctypes("/opt/axon/libaxon_pjrt.so")
            if hook is not None:
                m.set_axon_ntff_profile_hook(hook)
        except Exception:
            pass


def kernel(x, k):
    x = np.ascontiguousarray(np.asarray(x), dtype=np.float32)
    k = int(np.asarray(k))
    assert x.shape == (B, N, C), f"unexpected x shape {x.shape}"
    assert k == K, f"kernel compiled for k={K}, got {k}"

    _ensure_trace_safe()
    from concourse.bass_utils import run_bass_kernel_spmd

    if not _NC_CACHE:
        _NC_CACHE.append(_build_bass())
    nc = _NC_CACHE[0]
    res = run_bass_kernel_spmd(nc, _make_in_maps(x), core_ids=list(range(8)))
    kernel.last_results = res
    out = np.empty((B, N, N), np.float32)
    for core in range(8):
        b, half = divmod(core, 2)
        out[b, half * ROWS:(half + 1) * ROWS] = res.results[core]["outp"]
    return out
